# revision 11
# baseline (speedup 1.0000x reference)
"""Multi-head self-attention on 8 TRN2 NeuronCores.

Sharding: core c -> (batch b = c//2, head-half g = c%2, i.e. 8 of 16 heads).
Each core computes qkv-proj + attention + out-proj partial for its 8 heads;
host sums the two partials per batch and adds b_out.

All matmuls run in fp16 (1 cyc/row on the PE; psum accumulates fp32, and
fp16's 10-bit mantissa matches float32r). x stays resident in SBUF as fp16
so projections never re-DMA it. The attn@V matmul is restructured as
out[q=128, HD+1] = expT_chunk^T @ V_chunk accumulated over k-chunks: the
output occupies all 128 partitions with only 65 moving rows per matmul,
halving PE row count vs the [65, q] form. The appended ones column of V
lands the softmax denominator per-q-partition, so normalization is a
reciprocal + per-partition tensor_scalar multiply; the normalized ctx tile
[q, (hi,hd)] is PE-transposed (identity matmul) back to [ch, q] for the
out-projection. Biases are applied on the DVE (per-partition scalar add for
q/k, broadcast tensor add for v), not as PE ones-matmuls.

Schedule: exp on ACT is the critical path of the attention loop; next-pair
projections are dripped into each (pair, q-chunk) iteration via generators
so the PE stays fed between score/AV bursts.
"""
import sys
sys.path.insert(0, '/opt/trn_rl_repo')

import numpy as np

import concourse.bass as bass
import concourse.mybir as mybir
import concourse.tile as tile
from concourse import bacc

F32 = mybir.dt.float32
F16 = mybir.dt.float16

B, S, D = 4, 2048, 1024
H, HD = 16, 64            # total heads, head dim
HC = 8                    # heads per core
N_CORES = 8
SC = S // 512             # seq chunks of 512
EC = D // 128             # embed chunks of 128
NSK = S // 128            # sk chunks of 128


def build_nc(debug=False):
    nc = bacc.Bacc(None, target_bir_lowering=False)

    xT = nc.dram_tensor("xT", [D, S], F16, kind="ExternalInput")
    w_qk = nc.dram_tensor("w_qk", [D, 1024], F16, kind="ExternalInput")
    w_v = nc.dram_tensor("w_v", [D, 512], F16, kind="ExternalInput")
    b_qk = nc.dram_tensor("b_qk", [128, 8], F32, kind="ExternalInput")
    b_v = nc.dram_tensor("b_v", [128, 512], F32, kind="ExternalInput")
    ident = nc.dram_tensor("ident", [128, 128], F16, kind="ExternalInput")
    w_out = nc.dram_tensor("w_out", [512, D], F16, kind="ExternalInput")
    out = nc.dram_tensor("out", [S, D], F16, kind="ExternalOutput")
    if debug:
        dbg_pjt = nc.dram_tensor("dbg_pjt", [128, 2, S], F16, kind="ExternalOutput")
        dbg_ctxT = nc.dram_tensor("dbg_ctxT", [128, 4, S], F16, kind="ExternalOutput")
        dbg_v = nc.dram_tensor("dbg_v", [128, 16, 8, 65], F16, kind="ExternalOutput")
        dbg_expT = nc.dram_tensor("dbg_expT", [128, NSK, 512], F16, kind="ExternalOutput")
        dbg_ctxp = nc.dram_tensor("dbg_ctxp", [128, 4, HD + 1], F32, kind="ExternalOutput")

    with tile.TileContext(nc) as tc:
        with (
            tc.tile_pool(name="const", bufs=1) as cpool,
            tc.tile_pool(name="pjt", bufs=2) as pjt_pool,
            tc.tile_pool(name="vpool", bufs=1) as vpool,
            tc.tile_pool(name="outsb", bufs=2) as out_pool,
            tc.tile_pool(name="s1wq", bufs=2) as wq_pool,
            tc.tile_pool(name="ctxsb", bufs=2) as ctx_sb_pool,
            tc.tile_pool(name="rcp", bufs=2) as rc_pool,
            tc.tile_pool(name="expT", bufs=2) as expT_pool,
            tc.tile_pool(name="ctxT", bufs=1) as ctxT_pool,
            tc.tile_pool(name="s1ps", bufs=1, space="PSUM") as s1ps,
            tc.tile_pool(name="scps", bufs=2, space="PSUM") as sc_ps,
            tc.tile_pool(name="ctxps", bufs=2, space="PSUM") as ctx_ps,
            tc.tile_pool(name="tpps", bufs=1, space="PSUM") as tp_ps,
        ):
            w_out_sb = cpool.tile([128, 4, D], F16)
            nc.sync.dma_start(w_out_sb[:], w_out.rearrange("(c p) e -> p c e", p=128))
            b_qk_sb = cpool.tile([128, 8], F32)
            nc.sync.dma_start(b_qk_sb[:], b_qk[:])
            b_v_sb = cpool.tile([128, 512], F32)
            nc.sync.dma_start(b_v_sb[:], b_v[:])
            ident_sb = cpool.tile([128, 128], F16)
            nc.sync.dma_start(ident_sb[:], ident[:])
            w_v_sb = cpool.tile([128, EC, 512], F16)
            nc.sync.dma_start(w_v_sb[:], w_v.rearrange("(c p) f -> p c f", p=128))

            # resident x^T, chunked DMA so compute starts after 1 chunk
            xt = cpool.tile([128, EC, S], F16)
            for n in range(SC):
                nc.sync.dma_start(
                    xt[:, :, 512 * n:512 * (n + 1)],
                    xT.rearrange("(c p) s -> p c s", p=128)[:, :, 512 * n:512 * (n + 1)],
                )

            # v_sb[s % 128, s_tile, head, 0:64] = V; [..., 64] = 1.0
            v_sb = vpool.tile([128, S // 128, HC, HD + 1], F16)
            nc.vector.memset(v_sb[:, :, :, HD], 1.0)

            ctxT = ctxT_pool.tile([128, 4, S], F16)

            def v_chunk(n):
                for tl in range(4):
                    t = 4 * n + tl
                    ps = s1ps.tile([128, 512], F32, name="vps", tag="s1")
                    for ci in range(EC):
                        nc.tensor.matmul(
                            ps[:], xt[:, ci, 128 * t:128 * (t + 1)],
                            w_v_sb[:, ci, :],
                            start=(ci == 0), stop=(ci == EC - 1))
                    nc.vector.tensor_add(
                        v_sb[:, t, :, 0:HD],
                        ps.rearrange("p (h d) -> p h d", h=HC),
                        b_v_sb.rearrange("p (h d) -> p h d", h=HC))

            def stage1_pair_start(p):
                """Allocate tiles + weight DMA for pair p's q,k projection."""
                wq = wq_pool.tile([128, EC, 256], F16, name="wq", tag="wq")
                # columns 0:128 = q of pair p, 128:256 = k of pair p
                wsrc = w_qk.rearrange("(c p) f -> p c f", p=128)
                nc.sync.dma_start(wq[:, :, 0:128],
                                  wsrc[:, :, 128 * p:128 * (p + 1)])
                nc.sync.dma_start(wq[:, :, 128:256],
                                  wsrc[:, :, 512 + 128 * p:512 + 128 * (p + 1)])
                pjt = pjt_pool.tile([128, 2, S], F16, name="pjt", tag="pjt")
                return (pjt, wq)

            def gen_chunk(p, st, n, js):
                """q/k projection of pair p, seq chunk n, yielding every few
                matmuls so the caller can drip PE work under ACT-bound
                attention."""
                pjt, wq = st
                for j in js:
                    ps = s1ps.tile([128, 512], F32, name="qkps", tag="s1")
                    for ci in range(EC):
                        nc.tensor.matmul(
                            ps[:], wq[:, ci, 128 * j:128 * (j + 1)],
                            xt[:, ci, 512 * n:512 * (n + 1)],
                            start=(ci == 0), stop=(ci == EC - 1))
                        if ci % 4 == 3:
                            yield
                    nc.vector.tensor_scalar_add(
                        pjt[:, j, 512 * n:512 * (n + 1)], ps[:],
                        b_qk_sb[:, 2 * p + j:2 * p + j + 1])

            def chain(*gens):
                for g in gens:
                    yield from g

            def drain(g):
                for _ in g:
                    pass

            # ---- prologue: V projection interleaved with pair-0 k
            # projection; q chunk 0 of pair 0 last (q chunks 1-3 drip under
            # p=0 attention) ----
            st0 = stage1_pair_start(0)
            for n in range(SC):
                v_chunk(n)
                drain(gen_chunk(0, st0, n, (1,)))
            drain(gen_chunk(0, st0, 0, (0,)))
            pjts = {0: st0}

            # ---- attention, software-pipelined by one q-chunk: AV + norm +
            # transpose + out-proj of chunk i run under the ACT-bound exp
            # window of chunk i+1, and each AV accumulation chain runs
            # back-to-back within its PSUM bank (one group per bank) ----
            def make_tail(p, qc, expTs, ctxps):
                def tail():
                    # attn@V: out[q=128, qsub, 65] accumulated over the 16
                    # k-chunks; the ones column of V lands the denominator
                    for hi in range(2):
                        for qs in range(4):
                            for sk in range(NSK):
                                nc.tensor.matmul(
                                    ctxps[hi][:, qs, :],
                                    expTs[hi][:, sk, 128 * qs:128 * (qs + 1)],
                                    v_sb[:, sk, 2 * p + hi, :],
                                    start=(sk == 0), stop=(sk == NSK - 1))
                    if debug and p == 0 and qc == 0:
                        nc.sync.dma_start(dbg_expT[:], expTs[0][:])
                        dbg_sb = cpool.tile([128, 4, HD + 1], F32)
                        nc.vector.tensor_copy(dbg_sb[:], ctxps[0][:])
                        nc.sync.dma_start(dbg_ctxp[:], dbg_sb[:])

                    # normalization: reciprocal of col 64 + per-partition
                    # scalar multiply
                    ctx_sb = ctx_sb_pool.tile([128, 4, 128], F16,
                                              name="ctxsb", tag="ctxsb")
                    rcps = {}
                    for hi in range(2):
                        rcp = rc_pool.tile([128, 4], F32, name=f"rcp{hi}",
                                           tag="rcp")
                        nc.vector.reciprocal_approx_fast(
                            rcp[:], ctxps[hi][:, :, HD])
                        rcps[hi] = rcp
                    for qs in range(4):
                        for hi in range(2):
                            nc.vector.tensor_scalar_mul(
                                ctx_sb[:, qs, 64 * hi:64 * (hi + 1)],
                                ctxps[hi][:, qs, 0:HD],
                                rcps[hi][:, qs:qs + 1])

                    # transpose ctx [q, ch] -> ctxT [ch, q] via PE identity
                    for qs in range(4):
                        tq = 4 * qc + qs
                        tp = tp_ps.tile([128, 128], F16, name="tp", tag="tp")
                        nc.tensor.transpose(tp[:], ctx_sb[:, qs, :],
                                            ident_sb[:])
                        nc.vector.tensor_copy(
                            ctxT[:, p, 128 * tq:128 * (tq + 1)], tp[:])

                    # out-projection once the last pair's ctxT is in
                    if p == 3:
                        for tl in range(4):
                            tq = 4 * qc + tl
                            for ec in range(2):
                                ps4 = s1ps.tile([128, 512], F32,
                                                name="s4", tag="s1")
                                for pp in range(4):
                                    nc.tensor.matmul(
                                        ps4[:],
                                        ctxT[:, pp, 128 * tq:128 * (tq + 1)],
                                        w_out_sb[:, pp, 512 * ec:512 * (ec + 1)],
                                        start=(pp == 0), stop=(pp == 3))
                                o = out_pool.tile([128, 512], F16, name="o")
                                nc.vector.tensor_copy(o[:], ps4[:])
                                nc.sync.dma_start(
                                    out[128 * tq:128 * (tq + 1),
                                        512 * ec:512 * (ec + 1)], o[:])
                return tail

            pending = None
            for p in range(4):
                pjt = pjts[p][0]
                if debug and p == 1:
                    nc.sync.dma_start(dbg_pjt[:], pjts[0][0][:])
                    nc.sync.dma_start(dbg_v[:], v_sb[:])
                for qc in range(SC):
                    qsl = slice(512 * qc, 512 * (qc + 1))
                    expTs = {}
                    ctxps = {}
                    for hi in range(2):
                        expTs[hi] = expT_pool.tile([128, NSK, 512], F16,
                                                   name=f"expT{hi}", tag="expT")
                        ctxps[hi] = ctx_ps.tile([128, 4, HD + 1], F32,
                                                name=f"ctx{hi}", tag="ctx")

                    # PE work to drip under the ACT-bound grp loop
                    gens = []
                    if p == 0 and qc < SC - 1:
                        gens.append(gen_chunk(0, st0, qc + 1, (0,)))
                    if p < 3:
                        if qc == 0:
                            pjts[p + 1] = stage1_pair_start(p + 1)
                        gens.append(gen_chunk(p + 1, pjts[p + 1], qc, (0, 1)))
                    drip = chain(*gens)

                    for grp in range(NSK // 2):
                        for hi in range(2):
                            base = 64 * hi
                            scp = sc_ps.tile([128, 2, 512], F32, name="scp")
                            for gg in range(2):
                                sk = 2 * grp + gg
                                nc.tensor.matmul(
                                    scp[:, gg, :],
                                    pjt[base:base + 64, 1,
                                        128 * sk:128 * (sk + 1)],
                                    pjt[base:base + 64, 0, qsl],
                                    start=True, stop=True)
                            nc.scalar.activation(
                                expTs[hi][:, 2 * grp:2 * grp + 2, :],
                                scp[:],
                                mybir.ActivationFunctionType.Exp)
                        if grp == 1 and pending is not None:
                            pending()
                            pending = None
                        next(drip, None)
                        next(drip, None)
                    drain(drip)
                    pending = make_tail(p, qc, expTs, ctxps)
            pending()

            if debug:
                nc.sync.dma_start(dbg_ctxT[:], ctxT[:])

    nc.compile()
    return nc


# ---------------------------------------------------------------------------
# host side: shard, run SPMD, gather
# ---------------------------------------------------------------------------

_RUNNER = None


def _make_runner(nc, n_cores):
    """Jit-once SPMD runner via PJRT (axon)."""
    import jax
    from jax.sharding import Mesh, PartitionSpec
    from jax.experimental.shard_map import shard_map
    from concourse import bass2jax
    from concourse.bass2jax import _bass_exec_p, install_neuronx_cc_hook

    install_neuronx_cc_hook()
    partition_name = nc.partition_id_tensor.name if nc.partition_id_tensor else None

    in_names, out_names, out_avals, zero_outs = [], [], [], []
    for alloc in nc.m.functions[0].allocations:
        if not isinstance(alloc, mybir.MemoryLocationSet):
            continue
        name = alloc.memorylocations[0].name
        if alloc.kind == "ExternalInput":
            if name != partition_name:
                in_names.append(name)
        elif alloc.kind == "ExternalOutput":
            out_names.append(name)
            shape = tuple(alloc.tensor_shape)
            dtype = mybir.dt.np(alloc.dtype)
            out_avals.append(jax.core.ShapedArray(shape, dtype))
            zero_outs.append(np.zeros(shape, dtype))
    n_params = len(in_names)
    n_outs = len(out_avals)
    all_in_names = list(in_names) + list(out_names)
    if partition_name is not None:
        all_in_names.append(partition_name)

    def _body(*args):
        operands = list(args)
        if partition_name is not None:
            operands.append(bass2jax.partition_id_tensor())
        outs = _bass_exec_p.bind(
            *operands,
            out_avals=tuple(out_avals),
            in_names=tuple(all_in_names),
            out_names=tuple(out_names),
            lowering_input_output_aliases=(),
            sim_require_finite=True,
            sim_require_nnan=True,
            nc=nc,
        )
        return tuple(outs)

    devices = jax.devices()[:n_cores]
    if n_cores == 1:
        jitted = jax.jit(_body, keep_unused=True)

        def run1(in_maps):
            args = [np.asarray(in_maps[0][n]) for n in in_names] + list(zero_outs)
            out_arrs = jitted(*args)
            jax.block_until_ready(out_arrs)
            return [{n: np.asarray(out_arrs[i]) for i, n in enumerate(out_names)}]

        return run1

    mesh = Mesh(np.asarray(devices), ("core",))
    in_specs = (PartitionSpec("core"),) * (n_params + n_outs)
    out_specs = (PartitionSpec("core"),) * n_outs
    jitted = jax.jit(
        shard_map(_body, mesh=mesh, in_specs=in_specs, out_specs=out_specs,
                  check_rep=False),
        keep_unused=True,
    )

    def run(in_maps):
        concat_in = [
            np.concatenate([np.asarray(in_maps[c][n]) for c in range(n_cores)],
                           axis=0)
            for n in in_names
        ]
        concat_zero = [
            np.zeros((n_cores * z.shape[0], *z.shape[1:]), z.dtype)
            for z in zero_outs
        ]
        out_arrs = jitted(*concat_in, *concat_zero)
        jax.block_until_ready(out_arrs)
        return [
            {n: np.asarray(out_arrs[i]).reshape(n_cores, *out_avals[i].shape)[c]
             for i, n in enumerate(out_names)}
            for c in range(n_cores)
        ]

    return run


def _shard_inputs(qkv, W_in, b_in, W_out, b_out):
    """Build the 8 per-core input dicts."""
    x = np.asarray(qkv, np.float32)
    W_in = np.asarray(W_in, np.float32)
    b_in = np.asarray(b_in, np.float32)
    W_out = np.asarray(W_out, np.float32)
    scale = np.float32(1.0 / np.sqrt(HD))
    ident = np.eye(128, dtype=np.float16)

    in_maps = []
    for c in range(N_CORES):
        b, g = divmod(c, 2)
        qs = slice(512 * g, 512 * (g + 1))
        ks = slice(1024 + 512 * g, 1024 + 512 * (g + 1))
        vs = slice(2048 + 512 * g, 2048 + 512 * (g + 1))
        # b_qk col 2p+j = per-channel bias of pair p (j=0 q scaled, j=1 k)
        bq = (b_in[qs] * scale).reshape(4, 128)
        bk = b_in[ks].reshape(4, 128)
        b_qk2 = np.zeros((128, 8), np.float32)
        b_qk2[:, 0::2] = bq.T
        b_qk2[:, 1::2] = bk.T
        in_maps.append({
            "xT": np.ascontiguousarray(x[b].T).astype(np.float16),
            "w_qk": np.ascontiguousarray(
                np.concatenate([W_in[:, qs] * scale, W_in[:, ks]],
                               axis=1)).astype(np.float16),
            "w_v": np.ascontiguousarray(W_in[:, vs]).astype(np.float16),
            "b_qk": b_qk2,
            "b_v": np.ascontiguousarray(
                np.broadcast_to(b_in[vs], (128, 512))).astype(np.float32),
            "ident": ident,
            "w_out": np.ascontiguousarray(
                W_out[512 * g:512 * (g + 1), :]).astype(np.float16),
        })
    return in_maps


def kernel(qkv, W_in, b_in, W_out, b_out):
    global _RUNNER
    if _RUNNER is None:
        nc = build_nc()
        _RUNNER = _make_runner(nc, N_CORES)
    in_maps = _shard_inputs(qkv, W_in, b_in, W_out, b_out)
    results = _RUNNER(in_maps)
    b_out = np.asarray(b_out, np.float32)
    out = np.empty((B, S, D), np.float32)
    for b in range(B):
        out[b] = (results[2 * b]["out"].astype(np.float32)
                  + results[2 * b + 1]["out"].astype(np.float32) + b_out)
    return out


if __name__ == "__main__":
    rng = np.random.default_rng(0)
    qkv = rng.standard_normal((B, S, D)).astype(np.float32)
    sc = 1.0 / np.sqrt(D)
    W_in = rng.uniform(-sc, sc, (D, 3 * D)).astype(np.float32)
    b_in = rng.uniform(-sc, sc, (3 * D,)).astype(np.float32)
    W_out = rng.uniform(-sc, sc, (D, D)).astype(np.float32)
    b_out = rng.uniform(-sc, sc, (D,)).astype(np.float32)
    got = kernel(qkv, W_in, b_in, W_out, b_out)
    print("kernel ran, output shape", got.shape)


# revision 14
# speedup vs baseline: 1.0481x; 1.0481x over previous
"""Multi-head self-attention on 8 TRN2 NeuronCores.

Sharding: core c -> (batch b = c//2, head-half g = c%2, i.e. 8 of 16 heads).
Each core computes qkv-proj + attention + out-proj partial for its 8 heads;
host sums the two partials per batch and adds b_out.

All matmuls run in fp16 (1 cyc/row on the PE; psum accumulates fp32, and
fp16's 10-bit mantissa matches float32r). x stays resident in SBUF as fp16
so projections never re-DMA it. The attn@V matmul is restructured as
out[q=128, HD+1] = expT_chunk^T @ V_chunk accumulated over k-chunks: the
output occupies all 128 partitions with only 65 moving rows per matmul,
halving PE row count vs the [65, q] form. The appended ones column of V
lands the softmax denominator per-q-partition, so normalization is a
reciprocal + per-partition tensor_scalar multiply; the normalized ctx tile
[q, (hi,hd)] is PE-transposed (identity matmul) back to [ch, q] for the
out-projection. Biases are applied on the DVE (per-partition scalar add for
q/k, broadcast tensor add for v), not as PE ones-matmuls.

Schedule: exp on ACT is the critical path of the attention loop; next-pair
projections are dripped into each (pair, q-chunk) iteration via generators
so the PE stays fed between score/AV bursts.
"""
import sys
sys.path.insert(0, '/opt/trn_rl_repo')

import numpy as np

import concourse.bass as bass
import concourse.mybir as mybir
import concourse.tile as tile
from concourse import bacc

F32 = mybir.dt.float32
F16 = mybir.dt.float16

B, S, D = 4, 2048, 1024
H, HD = 16, 64            # total heads, head dim
HC = 8                    # heads per core
N_CORES = 8
SC = S // 512             # seq chunks of 512
EC = D // 128             # embed chunks of 128
NSK = S // 128            # sk chunks of 128


def build_nc(debug=False):
    nc = bacc.Bacc(None, target_bir_lowering=False)

    xT = nc.dram_tensor("xT", [D, S], F16, kind="ExternalInput")
    w_qk = nc.dram_tensor("w_qk", [D, 1024], F16, kind="ExternalInput")
    w_v = nc.dram_tensor("w_v", [D, 512], F16, kind="ExternalInput")
    b_qk = nc.dram_tensor("b_qk", [128, 8], F32, kind="ExternalInput")
    b_v = nc.dram_tensor("b_v", [128, 512], F32, kind="ExternalInput")
    ident = nc.dram_tensor("ident", [128, 128], F16, kind="ExternalInput")
    w_out = nc.dram_tensor("w_out", [512, D], F16, kind="ExternalInput")
    out = nc.dram_tensor("out", [S, D], F16, kind="ExternalOutput")
    if debug:
        dbg_pjt = nc.dram_tensor("dbg_pjt", [128, 2, S], F16, kind="ExternalOutput")
        dbg_ctxT = nc.dram_tensor("dbg_ctxT", [128, 4, S], F16, kind="ExternalOutput")
        dbg_v = nc.dram_tensor("dbg_v", [128, 16, 8, 65], F16, kind="ExternalOutput")
        dbg_expT = nc.dram_tensor("dbg_expT", [128, NSK, 512], F16, kind="ExternalOutput")
        dbg_ctxp = nc.dram_tensor("dbg_ctxp", [128, 4, HD + 1], F32, kind="ExternalOutput")

    with tile.TileContext(nc) as tc:
        with (
            tc.tile_pool(name="const", bufs=1) as cpool,
            tc.tile_pool(name="pjt", bufs=2) as pjt_pool,
            tc.tile_pool(name="vpool", bufs=1) as vpool,
            tc.tile_pool(name="outsb", bufs=2) as out_pool,
            tc.tile_pool(name="s1wq", bufs=2) as wq_pool,
            tc.tile_pool(name="ctxsb", bufs=2) as ctx_sb_pool,
            tc.tile_pool(name="rcp", bufs=2) as rc_pool,
            tc.tile_pool(name="expT", bufs=2) as expT_pool,
            tc.tile_pool(name="ctxT", bufs=1) as ctxT_pool,
            tc.tile_pool(name="s1ps", bufs=1, space="PSUM") as s1ps,
            tc.tile_pool(name="scps", bufs=2, space="PSUM") as sc_ps,
            tc.tile_pool(name="ctxps", bufs=2, space="PSUM") as ctx_ps,
            tc.tile_pool(name="tpps", bufs=1, space="PSUM") as tp_ps,
        ):
            # DMA order: x chunk 0 + pair-0 weights first so the PE starts
            # ASAP; w_out (needed only at the last pair) last.
            xt = cpool.tile([128, EC, S], F16)
            xsrc = xT.rearrange("(c p) s -> p c s", p=128)
            nc.sync.dma_start(xt[:, :, 0:512], xsrc[:, :, 0:512])
            wq_sbs = {}

            def stage1_pair_start(p):
                """Allocate tiles + weight DMA for pair p's q,k projection."""
                wq = wq_pool.tile([128, EC, 256], F16, name="wq", tag="wq")
                # columns 0:128 = q of pair p, 128:256 = k of pair p
                wsrc = w_qk.rearrange("(c p) f -> p c f", p=128)
                nc.sync.dma_start(wq[:, :, 0:128],
                                  wsrc[:, :, 128 * p:128 * (p + 1)])
                nc.sync.dma_start(wq[:, :, 128:256],
                                  wsrc[:, :, 512 + 128 * p:512 + 128 * (p + 1)])
                pjt = pjt_pool.tile([128, 2, S], F16, name="pjt", tag="pjt")
                return (pjt, wq)

            st0 = stage1_pair_start(0)
            b_qk_sb = cpool.tile([128, 8], F32)
            nc.sync.dma_start(b_qk_sb[:], b_qk[:])
            w_v_sb = cpool.tile([128, EC, 512], F16)
            nc.sync.dma_start(w_v_sb[:], w_v.rearrange("(c p) f -> p c f", p=128))
            for n in range(1, SC):
                nc.sync.dma_start(xt[:, :, 512 * n:512 * (n + 1)],
                                  xsrc[:, :, 512 * n:512 * (n + 1)])
            b_v_sb = cpool.tile([128, 512], F32)
            nc.sync.dma_start(b_v_sb[:], b_v[:])
            ident_sb = cpool.tile([128, 128], F16)
            nc.sync.dma_start(ident_sb[:], ident[:])
            w_out_sb = cpool.tile([128, 4, D], F16)
            nc.sync.dma_start(w_out_sb[:], w_out.rearrange("(c p) e -> p c e", p=128))

            # v_sb[s % 128, s_tile, head, 0:64] = V; [..., 64] = 1.0
            v_sb = vpool.tile([128, S // 128, HC, HD + 1], F16)
            nc.vector.memset(v_sb[:, :, :, HD], 1.0)

            ctxT = ctxT_pool.tile([128, 4, S], F16)

            def gen_v_chunk(n):
                """V projection of seq chunk n, yielding every 2 matmuls."""
                for tl in range(4):
                    t = 4 * n + tl
                    ps = s1ps.tile([128, 512], F32, name="vps", tag="s1")
                    for ci in range(EC):
                        nc.tensor.matmul(
                            ps[:], xt[:, ci, 128 * t:128 * (t + 1)],
                            w_v_sb[:, ci, :],
                            start=(ci == 0), stop=(ci == EC - 1))
                        if ci % 2 == 1:
                            yield
                    nc.vector.tensor_add(
                        v_sb[:, t, :, 0:HD],
                        ps.rearrange("p (h d) -> p h d", h=HC),
                        b_v_sb.rearrange("p (h d) -> p h d", h=HC))

            def gen_chunk(p, st, n, js):
                """q/k projection of pair p, seq chunk n, yielding every 2
                matmuls so the caller can drip PE work under ACT-bound
                attention."""
                pjt, wq = st
                for j in js:
                    ps = s1ps.tile([128, 512], F32, name="qkps", tag="s1")
                    for ci in range(EC):
                        nc.tensor.matmul(
                            ps[:], wq[:, ci, 128 * j:128 * (j + 1)],
                            xt[:, ci, 512 * n:512 * (n + 1)],
                            start=(ci == 0), stop=(ci == EC - 1))
                        if ci % 2 == 1:
                            yield
                    nc.vector.tensor_scalar_add(
                        pjt[:, j, 512 * n:512 * (n + 1)], ps[:],
                        b_qk_sb[:, 2 * p + j:2 * p + j + 1])

            def chain(*gens):
                for g in gens:
                    yield from g

            def drain(g):
                for _ in g:
                    pass

            # ---- minimal prologue: only k+q of pair-0 chunk 0; everything
            # else (pair-0 k1-3/q1, all V chunks) drips under (p0,qc0) ----
            drain(gen_chunk(0, st0, 0, (1,)))
            drain(gen_chunk(0, st0, 0, (0,)))
            pjts = {0: st0}

            # ---- attention, software-pipelined by one q-chunk: AV + norm +
            # transpose + out-proj of chunk i run under the ACT-bound exp
            # window of chunk i+1, and each AV accumulation chain runs
            # back-to-back within its PSUM bank (one group per bank) ----
            def make_tail(p, qc, expTs, ctxps):
                def tail():
                    # attn@V: out[q=128, qsub, 65] accumulated over the 16
                    # k-chunks; the ones column of V lands the denominator
                    for hi in range(2):
                        for qs in range(4):
                            for sk in range(NSK):
                                nc.tensor.matmul(
                                    ctxps[hi][:, qs, :],
                                    expTs[hi][:, sk, 128 * qs:128 * (qs + 1)],
                                    v_sb[:, sk, 2 * p + hi, :],
                                    start=(sk == 0), stop=(sk == NSK - 1))
                    if debug and p == 0 and qc == 0:
                        nc.sync.dma_start(dbg_expT[:], expTs[0][:])
                        dbg_sb = cpool.tile([128, 4, HD + 1], F32)
                        nc.vector.tensor_copy(dbg_sb[:], ctxps[0][:])
                        nc.sync.dma_start(dbg_ctxp[:], dbg_sb[:])

                    # normalization: reciprocal of col 64 + per-partition
                    # scalar multiply
                    ctx_sb = ctx_sb_pool.tile([128, 4, 128], F16,
                                              name="ctxsb", tag="ctxsb")
                    rcps = {}
                    for hi in range(2):
                        rcp = rc_pool.tile([128, 4], F32, name=f"rcp{hi}",
                                           tag="rcp")
                        nc.vector.reciprocal_approx_fast(
                            rcp[:], ctxps[hi][:, :, HD])
                        rcps[hi] = rcp
                    for qs in range(4):
                        for hi in range(2):
                            nc.vector.tensor_scalar_mul(
                                ctx_sb[:, qs, 64 * hi:64 * (hi + 1)],
                                ctxps[hi][:, qs, 0:HD],
                                rcps[hi][:, qs:qs + 1])

                    # transpose ctx [q, ch] -> ctxT [ch, q] via PE identity
                    for qs in range(4):
                        tq = 4 * qc + qs
                        tp = tp_ps.tile([128, 128], F16, name="tp", tag="tp")
                        nc.tensor.transpose(tp[:], ctx_sb[:, qs, :],
                                            ident_sb[:])
                        nc.vector.tensor_copy(
                            ctxT[:, p, 128 * tq:128 * (tq + 1)], tp[:])

                    # out-projection once the last pair's ctxT is in
                    if p == 3:
                        for tl in range(4):
                            tq = 4 * qc + tl
                            for ec in range(2):
                                ps4 = s1ps.tile([128, 512], F32,
                                                name="s4", tag="s1")
                                for pp in range(4):
                                    nc.tensor.matmul(
                                        ps4[:],
                                        ctxT[:, pp, 128 * tq:128 * (tq + 1)],
                                        w_out_sb[:, pp, 512 * ec:512 * (ec + 1)],
                                        start=(pp == 0), stop=(pp == 3))
                                o = out_pool.tile([128, 512], F16, name="o")
                                nc.vector.tensor_copy(o[:], ps4[:])
                                nc.sync.dma_start(
                                    out[128 * tq:128 * (tq + 1),
                                        512 * ec:512 * (ec + 1)], o[:])
                return tail

            pending = None
            for p in range(4):
                pjt = pjts[p][0]
                if debug and p == 1:
                    nc.sync.dma_start(dbg_pjt[:], pjts[0][0][:])
                    nc.sync.dma_start(dbg_v[:], v_sb[:])
                for qc in range(SC):
                    qsl = slice(512 * qc, 512 * (qc + 1))
                    expTs = {}
                    ctxps = {}
                    for hi in range(2):
                        expTs[hi] = expT_pool.tile([128, NSK, 512], F16,
                                                   name=f"expT{hi}", tag="expT")
                        ctxps[hi] = ctx_ps.tile([128, 4, HD + 1], F32,
                                                name=f"ctx{hi}", tag="ctx")

                    # PE work to drip under the ACT-bound grp loop. k chunks
                    # of the next pair land before its q chunks; a pair's
                    # q-chunk n+1 drips one q-chunk ahead of its use.
                    gens = []
                    if p == 0 and qc == 0:
                        # pair-0 k1-3 (needed by grp 2+ scores, in order),
                        # q1, then all V chunks (needed by the qc0 tail)
                        for n in (1, 2, 3):
                            gens.append(gen_chunk(0, st0, n, (1,)))
                        gens.append(gen_chunk(0, st0, 1, (0,)))
                        for n in range(SC):
                            gens.append(gen_v_chunk(n))
                        n_drip = 10
                    else:
                        if qc < SC - 1:
                            gens.append(gen_chunk(p, pjts[p], qc + 1, (0,)))
                        if p < 3:
                            if qc == 1:
                                pjts[p + 1] = stage1_pair_start(p + 1)
                            if qc >= 1:
                                nxt = pjts[p + 1]
                                ks = {1: (0, 1), 2: (2, 3)}.get(qc)
                                if ks is not None:
                                    for n in ks:
                                        gens.append(gen_chunk(p + 1, nxt, n, (1,)))
                                else:
                                    gens.append(gen_chunk(p + 1, nxt, 0, (0,)))
                        n_drip = 2
                    drip = chain(*gens)

                    for grp in range(NSK // 2):
                        for hi in range(2):
                            base = 64 * hi
                            scp = sc_ps.tile([128, 2, 512], F32, name="scp")
                            for gg in range(2):
                                sk = 2 * grp + gg
                                nc.tensor.matmul(
                                    scp[:, gg, :],
                                    pjt[base:base + 64, 1,
                                        128 * sk:128 * (sk + 1)],
                                    pjt[base:base + 64, 0, qsl],
                                    start=True, stop=True)
                            nc.scalar.activation(
                                expTs[hi][:, 2 * grp:2 * grp + 2, :],
                                scp[:],
                                mybir.ActivationFunctionType.Exp)
                        if grp == 1 and pending is not None:
                            pending()
                            pending = None
                        for _ in range(n_drip):
                            next(drip, None)
                    drain(drip)
                    pending = make_tail(p, qc, expTs, ctxps)
            pending()

            if debug:
                nc.sync.dma_start(dbg_ctxT[:], ctxT[:])

    nc.compile()
    return nc


# ---------------------------------------------------------------------------
# host side: shard, run SPMD, gather
# ---------------------------------------------------------------------------

_RUNNER = None


def _make_runner(nc, n_cores):
    """Jit-once SPMD runner via PJRT (axon)."""
    import jax
    from jax.sharding import Mesh, PartitionSpec
    from jax.experimental.shard_map import shard_map
    from concourse import bass2jax
    from concourse.bass2jax import _bass_exec_p, install_neuronx_cc_hook

    install_neuronx_cc_hook()
    partition_name = nc.partition_id_tensor.name if nc.partition_id_tensor else None

    in_names, out_names, out_avals, zero_outs = [], [], [], []
    for alloc in nc.m.functions[0].allocations:
        if not isinstance(alloc, mybir.MemoryLocationSet):
            continue
        name = alloc.memorylocations[0].name
        if alloc.kind == "ExternalInput":
            if name != partition_name:
                in_names.append(name)
        elif alloc.kind == "ExternalOutput":
            out_names.append(name)
            shape = tuple(alloc.tensor_shape)
            dtype = mybir.dt.np(alloc.dtype)
            out_avals.append(jax.core.ShapedArray(shape, dtype))
            zero_outs.append(np.zeros(shape, dtype))
    n_params = len(in_names)
    n_outs = len(out_avals)
    all_in_names = list(in_names) + list(out_names)
    if partition_name is not None:
        all_in_names.append(partition_name)

    def _body(*args):
        operands = list(args)
        if partition_name is not None:
            operands.append(bass2jax.partition_id_tensor())
        outs = _bass_exec_p.bind(
            *operands,
            out_avals=tuple(out_avals),
            in_names=tuple(all_in_names),
            out_names=tuple(out_names),
            lowering_input_output_aliases=(),
            sim_require_finite=True,
            sim_require_nnan=True,
            nc=nc,
        )
        return tuple(outs)

    devices = jax.devices()[:n_cores]
    if n_cores == 1:
        jitted = jax.jit(_body, keep_unused=True)

        def run1(in_maps):
            args = [np.asarray(in_maps[0][n]) for n in in_names] + list(zero_outs)
            out_arrs = jitted(*args)
            jax.block_until_ready(out_arrs)
            return [{n: np.asarray(out_arrs[i]) for i, n in enumerate(out_names)}]

        return run1

    mesh = Mesh(np.asarray(devices), ("core",))
    in_specs = (PartitionSpec("core"),) * (n_params + n_outs)
    out_specs = (PartitionSpec("core"),) * n_outs
    jitted = jax.jit(
        shard_map(_body, mesh=mesh, in_specs=in_specs, out_specs=out_specs,
                  check_rep=False),
        keep_unused=True,
    )

    def run(in_maps):
        concat_in = [
            np.concatenate([np.asarray(in_maps[c][n]) for c in range(n_cores)],
                           axis=0)
            for n in in_names
        ]
        concat_zero = [
            np.zeros((n_cores * z.shape[0], *z.shape[1:]), z.dtype)
            for z in zero_outs
        ]
        out_arrs = jitted(*concat_in, *concat_zero)
        jax.block_until_ready(out_arrs)
        return [
            {n: np.asarray(out_arrs[i]).reshape(n_cores, *out_avals[i].shape)[c]
             for i, n in enumerate(out_names)}
            for c in range(n_cores)
        ]

    return run


def _shard_inputs(qkv, W_in, b_in, W_out, b_out):
    """Build the 8 per-core input dicts."""
    x = np.asarray(qkv, np.float32)
    W_in = np.asarray(W_in, np.float32)
    b_in = np.asarray(b_in, np.float32)
    W_out = np.asarray(W_out, np.float32)
    scale = np.float32(1.0 / np.sqrt(HD))
    ident = np.eye(128, dtype=np.float16)

    in_maps = []
    for c in range(N_CORES):
        b, g = divmod(c, 2)
        qs = slice(512 * g, 512 * (g + 1))
        ks = slice(1024 + 512 * g, 1024 + 512 * (g + 1))
        vs = slice(2048 + 512 * g, 2048 + 512 * (g + 1))
        # b_qk col 2p+j = per-channel bias of pair p (j=0 q scaled, j=1 k)
        bq = (b_in[qs] * scale).reshape(4, 128)
        bk = b_in[ks].reshape(4, 128)
        b_qk2 = np.zeros((128, 8), np.float32)
        b_qk2[:, 0::2] = bq.T
        b_qk2[:, 1::2] = bk.T
        in_maps.append({
            "xT": np.ascontiguousarray(x[b].T).astype(np.float16),
            "w_qk": np.ascontiguousarray(
                np.concatenate([W_in[:, qs] * scale, W_in[:, ks]],
                               axis=1)).astype(np.float16),
            "w_v": np.ascontiguousarray(W_in[:, vs]).astype(np.float16),
            "b_qk": b_qk2,
            "b_v": np.ascontiguousarray(
                np.broadcast_to(b_in[vs], (128, 512))).astype(np.float32),
            "ident": ident,
            "w_out": np.ascontiguousarray(
                W_out[512 * g:512 * (g + 1), :]).astype(np.float16),
        })
    return in_maps


def kernel(qkv, W_in, b_in, W_out, b_out):
    global _RUNNER
    if _RUNNER is None:
        nc = build_nc()
        _RUNNER = _make_runner(nc, N_CORES)
    in_maps = _shard_inputs(qkv, W_in, b_in, W_out, b_out)
    results = _RUNNER(in_maps)
    b_out = np.asarray(b_out, np.float32)
    out = np.empty((B, S, D), np.float32)
    for b in range(B):
        out[b] = (results[2 * b]["out"].astype(np.float32)
                  + results[2 * b + 1]["out"].astype(np.float32) + b_out)
    return out


if __name__ == "__main__":
    rng = np.random.default_rng(0)
    qkv = rng.standard_normal((B, S, D)).astype(np.float32)
    sc = 1.0 / np.sqrt(D)
    W_in = rng.uniform(-sc, sc, (D, 3 * D)).astype(np.float32)
    b_in = rng.uniform(-sc, sc, (3 * D,)).astype(np.float32)
    W_out = rng.uniform(-sc, sc, (D, D)).astype(np.float32)
    b_out = rng.uniform(-sc, sc, (D,)).astype(np.float32)
    got = kernel(qkv, W_in, b_in, W_out, b_out)
    print("kernel ran, output shape", got.shape)
